# revision 4
# baseline (speedup 1.0000x reference)
"""ClusterPooling (gnn_message_passing) on 8 TRN2 NeuronCores.

Strategy (edge-sharded, per sharding hint):
  L0 (device, 8 cores): node features sharded N/8 per core; each core
      computes p = x_slice @ w[:C], q = x_slice @ w[C:] with the PE.
  host: per-edge score = tanh(p[src]+q[dst]+b) (O(E) scalar glue), exact
      min-label connected components, and index packing: scores are
      pre-aggregated by (cluster-label, dst) pair — x_out rows are
      X^T @ W with W the sparse per-(label,dst) weight matrix.
  L1 (device, 8 cores): the heavy phase. Entries sharded across cores;
      per 128-entry tile: dma_gather of x[dst] rows (512B each) from the
      replicated x table, then PE matmul with the stationary [128,32]
      score-placement matrix accumulating [32 labels x 128 feat] blocks
      in PSUM (partition-packed 3 tiles/bank at offsets 0/32/64).
  host: sum label blocks -> x_out rows; int outputs (cluster,
      edge_index_out, batch_out) assembled in numpy.
"""

import os
import time

import numpy as np

import concourse.bacc as bacc
import concourse.mybir as mybir
import concourse.tile as tile
from concourse.bass_utils import run_bass_kernel_spmd

F32 = mybir.dt.float32
I16 = mybir.dt.int16

NCORES = 8
P = 128
HALF = 32768          # int16 gather-index range -> lo/hi table split
TILES_PER_CHUNK = 8   # 1024 gather indices per DMAGather (SWDGE ring limit)
MAX_LABS = 32         # distinct labels per tile (PSUM offsets 0/32/64)


# ----------------------------------------------------------------- L0
def build_l0(n_slice, C=128):
    nc = bacc.Bacc("TRN2", target_bir_lowering=False, debug=False,
                   num_devices=NCORES)
    xT = nc.dram_tensor("xT", [C, n_slice], F32, kind="ExternalInput")
    w2 = nc.dram_tensor("w2", [C, 2], F32, kind="ExternalInput")
    pq = nc.dram_tensor("pq", [2, n_slice], F32, kind="ExternalOutput")
    CH = 512
    nch = (n_slice + CH - 1) // CH
    with tile.TileContext(nc) as tc:
        with (tc.tile_pool(name="sbuf", bufs=1) as pool,
              tc.tile_pool(name="psum", bufs=4, space="PSUM") as pp):
            w2_t = pool.tile([C, 2], F32)
            nc.sync.dma_start(out=w2_t[:], in_=w2[:, :])
            xt = pool.tile([C, n_slice], F32)
            nc.sync.dma_start(out=xt[:], in_=xT[:, :])
            ob = pool.tile([2, n_slice], F32)
            for i in range(nch):
                c0 = i * CH
                w = min(CH, n_slice - c0)
                ps = pp.tile([2, CH], F32, tag="ps")
                nc.tensor.matmul(out=ps[:, :w], lhsT=w2_t[:],
                                 rhs=xt[:, c0:c0 + w], start=True, stop=True)
                nc.vector.tensor_copy(out=ob[:, c0:c0 + w], in_=ps[:, :w])
            nc.sync.dma_start(out=pq[:, :], in_=ob[:])
    nc.compile()
    return nc


# ------------------------------------------------------------- host CC
def connected_components(src, dst, emask, n):
    """Exactly the reference algorithm (min-label propagation + pointer
    jumping); early exit at the fixed point it provably reaches."""
    lab = np.arange(n, dtype=np.int32)
    big = np.int32(n)
    iters = int(np.ceil(np.log2(max(n, 2)))) + 4
    for _ in range(iters):
        m = np.minimum(lab[src], lab[dst])
        upd = np.where(emask, m, big).astype(np.int32)
        prev = lab
        lab2 = lab.copy()
        np.minimum.at(lab2, src, upd)
        np.minimum.at(lab2, dst, upd)
        lab = lab2[lab2]
        if np.array_equal(lab, prev):
            break
    return lab


# -------------------------------------------------------------- L1 plan
class _Plan:
    pass


def build_l1_plan(dst, score, lab_edge, n_labels, N, singles=None,
                  tiles_per_chunk=TILES_PER_CHUNK, max_labs=MAX_LABS,
                  dedup=True):
    if singles is not None and len(singles[0]):
        dst = np.concatenate([dst, singles[0]])
        score = np.concatenate([score, np.ones(len(singles[0]), np.float32)])
        lab_edge = np.concatenate([lab_edge, singles[1]])
    key = lab_edge.astype(np.int64) * N + dst.astype(np.int64)
    if dedup:
        uk, inv = np.unique(key, return_inverse=True)
        wv = np.zeros(len(uk), np.float64)
        np.add.at(wv, inv, score.astype(np.float64))
        ent_lab = (uk // N).astype(np.int64)
        ent_v = (uk % N).astype(np.int64)
        ent_w = wv.astype(np.float32)
    else:
        order = np.argsort(key, kind="stable")
        ent_lab = lab_edge[order].astype(np.int64)
        ent_v = dst[order].astype(np.int64)
        ent_w = score[order].astype(np.float32)

    plan = _Plan()
    plan.max_labs = max_labs
    plan.tiles_per_chunk = tiles_per_chunk
    plan.n_labels = n_labels

    def make_tiles(v, w, lb):
        n = len(v)
        idx_t, ss_t, labs_t = [], [], []
        i = 0
        while i < n:
            jend = min(i + P, n)
            seg_lab = lb[i:jend]
            chg = np.flatnonzero(np.diff(seg_lab)) + 1
            starts = np.concatenate([[0], chg])
            if len(starts) > max_labs:
                jend = i + starts[max_labs]
                seg_lab = lb[i:jend]
                chg = np.flatnonzero(np.diff(seg_lab)) + 1
                starts = np.concatenate([[0], chg])
            k = jend - i
            it = np.zeros(P, np.int64)
            it[:k] = v[i:jend]
            ss = np.zeros((P, max_labs), np.float32)
            col = np.searchsorted(starts, np.arange(k), side="right") - 1
            ss[np.arange(k), col] = w[i:jend]
            labs = np.full(max_labs, -1, np.int64)
            ulabs = seg_lab[starts]
            labs[:len(ulabs)] = ulabs
            idx_t.append(it)
            ss_t.append(ss)
            labs_t.append(labs)
            i = jend
        if not idx_t:
            return (np.zeros((0, P), np.int64),
                    np.zeros((0, P, max_labs), np.float32),
                    np.zeros((0, max_labs), np.int64))
        return np.array(idx_t), np.array(ss_t), np.array(labs_t)

    hi = ent_v >= HALF
    lo_idx, lo_ss, lo_labs = make_tiles(ent_v[~hi], ent_w[~hi], ent_lab[~hi])
    hi_idx, hi_ss, hi_labs = make_tiles(ent_v[hi] - HALF, ent_w[hi],
                                        ent_lab[hi])

    def shard(ti, ts, tl):
        T = len(ti)
        Tc = -(-T // NCORES) if T else 0
        if Tc:
            Tc = -(-Tc // tiles_per_chunk) * tiles_per_chunk
        else:
            Tc = tiles_per_chunk
        padT = Tc * NCORES - T
        if padT:
            pad_i = np.zeros((padT, P), np.int64)
            pad_s = np.zeros((padT, P, max_labs), np.float32)
            pad_l = np.full((padT, max_labs), -1)
            ti = np.vstack([ti, pad_i]) if len(ti) else pad_i
            ts = np.vstack([ts, pad_s]) if len(ts) else pad_s
            tl = np.vstack([tl, pad_l]) if len(tl) else pad_l
        return (ti.reshape(NCORES, Tc, P),
                ts.reshape(NCORES, Tc, P, max_labs),
                tl.reshape(NCORES, Tc, max_labs), Tc)

    plan.lo_idx, plan.lo_ss, plan.lo_labs, plan.T_lo = shard(lo_idx, lo_ss,
                                                             lo_labs)
    plan.hi_idx, plan.hi_ss, plan.hi_labs, plan.T_hi = shard(hi_idx, hi_ss,
                                                             hi_labs)
    plan.n_tiles = plan.T_lo + plan.T_hi
    return plan


def pack_idx16(idx_tiles):
    flat = idx_tiles.reshape(-1)
    n = flat.shape[0]
    w = np.zeros((16, n // 16), np.int16)
    ar = np.arange(n)
    w[ar % 16, ar // 16] = flat.astype(np.int16)
    return np.tile(w, (8, 1))


# ------------------------------------------------------------ L1 builder
def build_l1(N, n_tiles_lo, n_tiles_hi, tiles_per_chunk=TILES_PER_CHUNK,
             C=128, half=HALF, max_labs=MAX_LABS):
    assert max_labs == 32
    T = n_tiles_lo + n_tiles_hi
    SLOTS = tiles_per_chunk * P
    n_chunks = T // tiles_per_chunk
    n_grp = (tiles_per_chunk + 2) // 3
    nc = bacc.Bacc("TRN2", target_bir_lowering=False, debug=False,
                   num_devices=NCORES, num_swdge_queues=4)
    x = nc.dram_tensor("x", [N, C], F32, kind="ExternalInput")
    idx16 = nc.dram_tensor("idx16", [P, T * P // 16], I16,
                           kind="ExternalInput")
    ss = nc.dram_tensor("ss", [P, T * max_labs], F32, kind="ExternalInput")
    tbl = nc.dram_tensor("tbl", [P, n_chunks * n_grp * C], F32,
                         kind="ExternalOutput")
    n_chunks_lo = n_tiles_lo // tiles_per_chunk
    icpc = SLOTS // 16
    with tile.TileContext(nc) as tc:
        with (tc.tile_pool(name="gat", bufs=4) as gat_pool,
              tc.tile_pool(name="idxp", bufs=1) as idx_pool,
              tc.tile_pool(name="ssp", bufs=1) as ss_pool,
              tc.tile_pool(name="obp", bufs=4) as ob_pool,
              tc.tile_pool(name="psum", bufs=6, space="PSUM") as pp):
            it = idx_pool.tile([P, T * P // 16], I16)
            nc.sync.dma_start(out=it[:], in_=idx16[:, :])
            sst = ss_pool.tile([P, T * max_labs], F32)
            nc.sync.dma_start(out=sst[:], in_=ss[:, :])
            HALFSLOTS = SLOTS // 2
            for chunk in range(n_chunks):
                tab = x[:half, :] if chunk < n_chunks_lo else x[half:, :]
                g = gat_pool.tile([P, tiles_per_chunk, C], F32, tag="g")
                for hh in range(2):
                    nc.gpsimd.dma_gather(
                        out_ap=g[:, hh * (tiles_per_chunk // 2):
                                 (hh + 1) * (tiles_per_chunk // 2), :],
                        in_ap=tab,
                        idxs_ap=it[:, chunk * (SLOTS // 16) +
                                   hh * (HALFSLOTS // 16):
                                   chunk * (SLOTS // 16) +
                                   (hh + 1) * (HALFSLOTS // 16)],
                        num_idxs=HALFSLOTS, num_idxs_reg=HALFSLOTS,
                        elem_size=C, queue_num=(2 * chunk + hh) % 4)
                sbase = chunk * tiles_per_chunk * max_labs
                for grp in range(n_grp):
                    j0, j1 = grp * 3, min(grp * 3 + 3, tiles_per_chunk)
                    ps = pp.tile([P, C], F32, tag="ps")
                    for j in range(j0, j1):
                        off = (j - j0) * 32
                        nc.tensor.matmul(
                            out=ps[off:off + max_labs, :],
                            lhsT=sst[:, sbase + j * max_labs:
                                     sbase + (j + 1) * max_labs],
                            rhs=g[:, j, :],
                            start=True, stop=True)
                    ob = ob_pool.tile([P, C], F32, tag="ob")
                    ru = (j1 - j0) * 32
                    nc.vector.tensor_copy(out=ob[:ru, :], in_=ps[:ru, :])
                    nc.sync.dma_start(
                        out=tbl[:ru, (chunk * n_grp + grp) * C:
                                (chunk * n_grp + grp + 1) * C],
                        in_=ob[:ru, :])
    nc.compile()
    return nc


def unscramble(tbls, plan, n_labels, C=128):
    rows = np.zeros((n_labels, C), np.float64)
    tpc = plan.tiles_per_chunk
    ml = plan.max_labs
    n_grp = (tpc + 2) // 3
    for c in range(NCORES):
        tbl = tbls[c].astype(np.float64)
        for t in range(plan.n_tiles):
            labs = (plan.lo_labs[c][t] if t < plan.T_lo
                    else plan.hi_labs[c][t - plan.T_lo])
            valid = labs >= 0
            if not valid.any():
                continue
            k, j = divmod(t, tpc)
            grp, pos = divmod(j, 3)
            col0 = (k * n_grp + grp) * C
            blk = tbl[pos * 32:pos * 32 + ml, col0:col0 + C]
            np.add.at(rows, labs[valid], blk[valid])
    return rows


# ---------------------------------------------------------------- kernel
def kernel(x, edge_index, batch, w, b):
    trace = os.environ.get("KERNEL_TRACE", "0") == "1"
    x = np.ascontiguousarray(np.asarray(x, dtype=np.float32))
    edge_index = np.asarray(edge_index)
    batch = np.asarray(batch)
    w = np.asarray(w, dtype=np.float32).reshape(-1)
    b = np.asarray(b, dtype=np.float32).reshape(-1)
    N, C = x.shape
    exec_ns = []

    # L0: p, q
    n_slice = N // NCORES
    assert n_slice * NCORES == N
    nc0 = build_l0(n_slice, C)
    w2 = np.ascontiguousarray(np.stack([w[:C], w[C:]], axis=1))
    in_maps = []
    for c in range(NCORES):
        xT = np.ascontiguousarray(x[c * n_slice:(c + 1) * n_slice].T)
        in_maps.append({"xT": xT, "w2": w2})
    r0 = run_bass_kernel_spmd(nc0, in_maps, core_ids=list(range(NCORES)),
                              trace=trace)
    if r0.exec_time_ns:
        exec_ns.append(("L0", r0.exec_time_ns))
    p = np.concatenate([r0.results[c]["pq"][0] for c in range(NCORES)])
    q = np.concatenate([r0.results[c]["pq"][1] for c in range(NCORES)])

    # host: scores, CC, plan
    src = edge_index[0].astype(np.int64)
    dst = edge_index[1].astype(np.int64)
    score = np.tanh(p[src] + q[dst] + np.float32(b[0])).astype(np.float32)
    contract = score > 0.0
    cluster = connected_components(src, dst, contract, N)
    cnt = np.bincount(cluster, minlength=N)
    single = cnt[cluster] == 1
    label_vals, lab_of_node = np.unique(cluster, return_inverse=True)
    n_labels = len(label_vals)
    sing_nodes = np.flatnonzero(single)
    plan = build_l1_plan(dst, score, lab_of_node[src], n_labels, N,
                         singles=(sing_nodes, lab_of_node[sing_nodes]))

    # L1: gather + weighted label-block matmuls
    nc1 = build_l1(N, plan.T_lo, plan.T_hi,
                   tiles_per_chunk=plan.tiles_per_chunk,
                   max_labs=plan.max_labs)
    in_maps = []
    for c in range(NCORES):
        idx_all = np.concatenate([pack_idx16(plan.lo_idx[c]),
                                  pack_idx16(plan.hi_idx[c])], axis=1)
        ss_all = np.concatenate([plan.lo_ss[c], plan.hi_ss[c]], axis=0)
        ss_flat = np.ascontiguousarray(
            ss_all.transpose(1, 0, 2).reshape(P, -1))
        in_maps.append({"x": x, "idx16": idx_all, "ss": ss_flat})
    r1 = run_bass_kernel_spmd(nc1, in_maps, core_ids=list(range(NCORES)),
                              trace=trace)
    if r1.exec_time_ns:
        exec_ns.append(("L1", r1.exec_time_ns))
    tbls = [r1.results[c]["tbl"] for c in range(NCORES)]

    # host: assemble outputs
    rows = unscramble(tbls, plan, n_labels, C)
    x_out = np.zeros((N, C), np.float32)
    x_out[label_vals] = rows.astype(np.float32)
    eio = np.stack([cluster[src], cluster[dst]]).astype(np.int32)
    batch_out = np.zeros((N,), batch.dtype)
    batch_out[cluster] = batch
    kernel.last_exec_ns = exec_ns
    return x_out, eio, batch_out, cluster.astype(np.int32)


# revision 5
# speedup vs baseline: 1.1244x; 1.1244x over previous
"""ClusterPooling (gnn_message_passing) on 8 TRN2 NeuronCores.

Strategy (edge-sharded, per sharding hint):
  L0 (device, 8 cores): node features sharded N/8 per core; each core
      computes p = x_slice @ w[:C], q = x_slice @ w[C:] with the PE.
  host: per-edge score = tanh(p[src]+q[dst]+b) (O(E) scalar glue), exact
      min-label connected components, and index packing: scores are
      pre-aggregated by (cluster-label, dst) pair — x_out rows are
      X^T @ W with W the sparse per-(label,dst) weight matrix.
  L1 (device, 8 cores): the heavy phase. Entries sharded across cores;
      per 128-entry tile: dma_gather of x[dst] rows (512B each) from the
      replicated x table, then PE matmul with the stationary [128,32]
      score-placement matrix accumulating [32 labels x 128 feat] blocks
      in PSUM (partition-packed 3 tiles/bank at offsets 0/32/64).
  host: sum label blocks -> x_out rows; int outputs (cluster,
      edge_index_out, batch_out) assembled in numpy.
"""

import os
import time

import numpy as np

import concourse.bacc as bacc
import concourse.mybir as mybir
import concourse.tile as tile
from concourse.bass_utils import run_bass_kernel_spmd

F32 = mybir.dt.float32
I16 = mybir.dt.int16

NCORES = 8
P = 128
HALF = 32768          # int16 gather-index range -> lo/hi table split
TILES_PER_CHUNK = 8   # 1024 gather indices per DMAGather (SWDGE ring limit)
MAX_LABS = 32         # distinct labels per tile (PSUM offsets 0/32/64)


# ----------------------------------------------------------------- L0
def build_l0(n_slice, C=128):
    nc = bacc.Bacc("TRN2", target_bir_lowering=False, debug=False,
                   num_devices=NCORES)
    xT = nc.dram_tensor("xT", [C, n_slice], F32, kind="ExternalInput")
    w2 = nc.dram_tensor("w2", [C, 2], F32, kind="ExternalInput")
    pq = nc.dram_tensor("pq", [2, n_slice], F32, kind="ExternalOutput")
    CH = 512
    nch = (n_slice + CH - 1) // CH
    with tile.TileContext(nc) as tc:
        with (tc.tile_pool(name="sbuf", bufs=1) as pool,
              tc.tile_pool(name="psum", bufs=4, space="PSUM") as pp):
            w2_t = pool.tile([C, 2], F32)
            nc.sync.dma_start(out=w2_t[:], in_=w2[:, :])
            xt = pool.tile([C, n_slice], F32)
            ob = pool.tile([2, n_slice], F32)
            for i in range(nch):
                c0 = i * CH
                w = min(CH, n_slice - c0)
                nc.sync.dma_start(out=xt[:, c0:c0 + w], in_=xT[:, c0:c0 + w])
                ps = pp.tile([2, CH], F32, tag="ps")
                nc.tensor.matmul(out=ps[:, :w], lhsT=w2_t[:],
                                 rhs=xt[:, c0:c0 + w], start=True, stop=True)
                nc.vector.tensor_copy(out=ob[:, c0:c0 + w], in_=ps[:, :w])
            nc.sync.dma_start(out=pq[:, :], in_=ob[:])
    nc.compile()
    return nc


# ------------------------------------------------------------- host CC
def connected_components(src, dst, emask, n):
    """Exactly the reference algorithm (min-label propagation + pointer
    jumping); early exit at the fixed point it provably reaches."""
    lab = np.arange(n, dtype=np.int32)
    big = np.int32(n)
    iters = int(np.ceil(np.log2(max(n, 2)))) + 4
    for _ in range(iters):
        m = np.minimum(lab[src], lab[dst])
        upd = np.where(emask, m, big).astype(np.int32)
        prev = lab
        lab2 = lab.copy()
        np.minimum.at(lab2, src, upd)
        np.minimum.at(lab2, dst, upd)
        lab = lab2[lab2]
        if np.array_equal(lab, prev):
            break
    return lab


# -------------------------------------------------------------- L1 plan
class _Plan:
    pass


def build_l1_plan(dst, score, lab_edge, n_labels, N, singles=None,
                  tiles_per_chunk=TILES_PER_CHUNK, max_labs=MAX_LABS,
                  dedup=True):
    if singles is not None and len(singles[0]):
        dst = np.concatenate([dst, singles[0]])
        score = np.concatenate([score, np.ones(len(singles[0]), np.float32)])
        lab_edge = np.concatenate([lab_edge, singles[1]])
    key = lab_edge.astype(np.int64) * N + dst.astype(np.int64)
    if dedup:
        uk, inv = np.unique(key, return_inverse=True)
        wv = np.zeros(len(uk), np.float64)
        np.add.at(wv, inv, score.astype(np.float64))
        ent_lab = (uk // N).astype(np.int64)
        ent_v = (uk % N).astype(np.int64)
        ent_w = wv.astype(np.float32)
    else:
        order = np.argsort(key, kind="stable")
        ent_lab = lab_edge[order].astype(np.int64)
        ent_v = dst[order].astype(np.int64)
        ent_w = score[order].astype(np.float32)

    plan = _Plan()
    plan.max_labs = max_labs
    plan.tiles_per_chunk = tiles_per_chunk
    plan.n_labels = n_labels

    def make_tiles(v, w, lb):
        n = len(v)
        idx_t, ss_t, labs_t = [], [], []
        i = 0
        while i < n:
            jend = min(i + P, n)
            seg_lab = lb[i:jend]
            chg = np.flatnonzero(np.diff(seg_lab)) + 1
            starts = np.concatenate([[0], chg])
            if len(starts) > max_labs:
                jend = i + starts[max_labs]
                seg_lab = lb[i:jend]
                chg = np.flatnonzero(np.diff(seg_lab)) + 1
                starts = np.concatenate([[0], chg])
            k = jend - i
            it = np.zeros(P, np.int64)
            it[:k] = v[i:jend]
            ss = np.zeros((P, max_labs), np.float32)
            col = np.searchsorted(starts, np.arange(k), side="right") - 1
            ss[np.arange(k), col] = w[i:jend]
            labs = np.full(max_labs, -1, np.int64)
            ulabs = seg_lab[starts]
            labs[:len(ulabs)] = ulabs
            idx_t.append(it)
            ss_t.append(ss)
            labs_t.append(labs)
            i = jend
        if not idx_t:
            return (np.zeros((0, P), np.int64),
                    np.zeros((0, P, max_labs), np.float32),
                    np.zeros((0, max_labs), np.int64))
        return np.array(idx_t), np.array(ss_t), np.array(labs_t)

    hi = ent_v >= HALF
    lo_idx, lo_ss, lo_labs = make_tiles(ent_v[~hi], ent_w[~hi], ent_lab[~hi])
    hi_idx, hi_ss, hi_labs = make_tiles(ent_v[hi] - HALF, ent_w[hi],
                                        ent_lab[hi])

    def shard(ti, ts, tl):
        T = len(ti)
        Tc = -(-T // NCORES) if T else 0
        if Tc:
            Tc = -(-Tc // tiles_per_chunk) * tiles_per_chunk
        else:
            Tc = tiles_per_chunk
        padT = Tc * NCORES - T
        if padT:
            pad_i = np.zeros((padT, P), np.int64)
            pad_s = np.zeros((padT, P, max_labs), np.float32)
            pad_l = np.full((padT, max_labs), -1)
            ti = np.vstack([ti, pad_i]) if len(ti) else pad_i
            ts = np.vstack([ts, pad_s]) if len(ts) else pad_s
            tl = np.vstack([tl, pad_l]) if len(tl) else pad_l
        return (ti.reshape(NCORES, Tc, P),
                ts.reshape(NCORES, Tc, P, max_labs),
                tl.reshape(NCORES, Tc, max_labs), Tc)

    plan.lo_idx, plan.lo_ss, plan.lo_labs, plan.T_lo = shard(lo_idx, lo_ss,
                                                             lo_labs)
    plan.hi_idx, plan.hi_ss, plan.hi_labs, plan.T_hi = shard(hi_idx, hi_ss,
                                                             hi_labs)
    plan.n_tiles = plan.T_lo + plan.T_hi
    return plan


def pack_idx16(idx_tiles):
    flat = idx_tiles.reshape(-1)
    n = flat.shape[0]
    w = np.zeros((16, n // 16), np.int16)
    ar = np.arange(n)
    w[ar % 16, ar // 16] = flat.astype(np.int16)
    return np.tile(w, (8, 1))


# ------------------------------------------------------------ L1 builder
def build_l1(N, n_tiles_lo, n_tiles_hi, tiles_per_chunk=TILES_PER_CHUNK,
             C=128, half=HALF, max_labs=MAX_LABS):
    assert max_labs == 32
    T = n_tiles_lo + n_tiles_hi
    SLOTS = tiles_per_chunk * P
    n_chunks = T // tiles_per_chunk
    n_grp = (tiles_per_chunk + 2) // 3
    nc = bacc.Bacc("TRN2", target_bir_lowering=False, debug=False,
                   num_devices=NCORES, num_swdge_queues=4)
    x = nc.dram_tensor("x", [N, C], F32, kind="ExternalInput")
    idx16 = nc.dram_tensor("idx16", [P, T * P // 16], I16,
                           kind="ExternalInput")
    ss = nc.dram_tensor("ss", [P, T * max_labs], F32, kind="ExternalInput")
    tbl = nc.dram_tensor("tbl", [P, n_chunks * n_grp * C], F32,
                         kind="ExternalOutput")
    n_chunks_lo = n_tiles_lo // tiles_per_chunk
    icpc = SLOTS // 16
    with tile.TileContext(nc) as tc:
        with (tc.tile_pool(name="gat", bufs=4) as gat_pool,
              tc.tile_pool(name="idxp", bufs=1) as idx_pool,
              tc.tile_pool(name="ssp", bufs=1) as ss_pool,
              tc.tile_pool(name="obp", bufs=4) as ob_pool,
              tc.tile_pool(name="psum", bufs=6, space="PSUM") as pp):
            icpc = SLOTS // 16
            sspc = tiles_per_chunk * max_labs
            it = idx_pool.tile([P, T * P // 16], I16)
            sst = ss_pool.tile([P, T * max_labs], F32)
            for chunk in range(n_chunks):
                nc.sync.dma_start(
                    out=it[:, chunk * icpc:(chunk + 1) * icpc],
                    in_=idx16[:, chunk * icpc:(chunk + 1) * icpc])
            for chunk in range(n_chunks):
                nc.sync.dma_start(
                    out=sst[:, chunk * sspc:(chunk + 1) * sspc],
                    in_=ss[:, chunk * sspc:(chunk + 1) * sspc])
            HALFSLOTS = SLOTS // 2
            for chunk in range(n_chunks):
                tab = x[:half, :] if chunk < n_chunks_lo else x[half:, :]
                g = gat_pool.tile([P, tiles_per_chunk, C], F32, tag="g")
                for hh in range(2):
                    nc.gpsimd.dma_gather(
                        out_ap=g[:, hh * (tiles_per_chunk // 2):
                                 (hh + 1) * (tiles_per_chunk // 2), :],
                        in_ap=tab,
                        idxs_ap=it[:, chunk * (SLOTS // 16) +
                                   hh * (HALFSLOTS // 16):
                                   chunk * (SLOTS // 16) +
                                   (hh + 1) * (HALFSLOTS // 16)],
                        num_idxs=HALFSLOTS, num_idxs_reg=HALFSLOTS,
                        elem_size=C, queue_num=(2 * chunk + hh) % 4)
                sbase = chunk * tiles_per_chunk * max_labs
                for grp in range(n_grp):
                    j0, j1 = grp * 3, min(grp * 3 + 3, tiles_per_chunk)
                    ps = pp.tile([P, C], F32, tag="ps")
                    for j in range(j0, j1):
                        off = (j - j0) * 32
                        nc.tensor.matmul(
                            out=ps[off:off + max_labs, :],
                            lhsT=sst[:, sbase + j * max_labs:
                                     sbase + (j + 1) * max_labs],
                            rhs=g[:, j, :],
                            start=True, stop=True)
                    ob = ob_pool.tile([P, C], F32, tag="ob")
                    ru = (j1 - j0) * 32
                    nc.vector.tensor_copy(out=ob[:ru, :], in_=ps[:ru, :])
                    nc.sync.dma_start(
                        out=tbl[:ru, (chunk * n_grp + grp) * C:
                                (chunk * n_grp + grp + 1) * C],
                        in_=ob[:ru, :])
    nc.compile()
    return nc


def unscramble(tbls, plan, n_labels, C=128):
    rows = np.zeros((n_labels, C), np.float64)
    tpc = plan.tiles_per_chunk
    ml = plan.max_labs
    n_grp = (tpc + 2) // 3
    for c in range(NCORES):
        tbl = tbls[c].astype(np.float64)
        for t in range(plan.n_tiles):
            labs = (plan.lo_labs[c][t] if t < plan.T_lo
                    else plan.hi_labs[c][t - plan.T_lo])
            valid = labs >= 0
            if not valid.any():
                continue
            k, j = divmod(t, tpc)
            grp, pos = divmod(j, 3)
            col0 = (k * n_grp + grp) * C
            blk = tbl[pos * 32:pos * 32 + ml, col0:col0 + C]
            np.add.at(rows, labs[valid], blk[valid])
    return rows


# ---------------------------------------------------------------- kernel
def kernel(x, edge_index, batch, w, b):
    trace = os.environ.get("KERNEL_TRACE", "0") == "1"
    x = np.ascontiguousarray(np.asarray(x, dtype=np.float32))
    edge_index = np.asarray(edge_index)
    batch = np.asarray(batch)
    w = np.asarray(w, dtype=np.float32).reshape(-1)
    b = np.asarray(b, dtype=np.float32).reshape(-1)
    N, C = x.shape
    exec_ns = []

    # L0: p, q
    n_slice = N // NCORES
    assert n_slice * NCORES == N
    nc0 = build_l0(n_slice, C)
    w2 = np.ascontiguousarray(np.stack([w[:C], w[C:]], axis=1))
    in_maps = []
    for c in range(NCORES):
        xT = np.ascontiguousarray(x[c * n_slice:(c + 1) * n_slice].T)
        in_maps.append({"xT": xT, "w2": w2})
    r0 = run_bass_kernel_spmd(nc0, in_maps, core_ids=list(range(NCORES)),
                              trace=trace)
    if r0.exec_time_ns:
        exec_ns.append(("L0", r0.exec_time_ns))
    p = np.concatenate([r0.results[c]["pq"][0] for c in range(NCORES)])
    q = np.concatenate([r0.results[c]["pq"][1] for c in range(NCORES)])

    # host: scores, CC, plan
    src = edge_index[0].astype(np.int64)
    dst = edge_index[1].astype(np.int64)
    score = np.tanh(p[src] + q[dst] + np.float32(b[0])).astype(np.float32)
    contract = score > 0.0
    cluster = connected_components(src, dst, contract, N)
    cnt = np.bincount(cluster, minlength=N)
    single = cnt[cluster] == 1
    label_vals, lab_of_node = np.unique(cluster, return_inverse=True)
    n_labels = len(label_vals)
    sing_nodes = np.flatnonzero(single)
    plan = build_l1_plan(dst, score, lab_of_node[src], n_labels, N,
                         singles=(sing_nodes, lab_of_node[sing_nodes]))

    # L1: gather + weighted label-block matmuls
    nc1 = build_l1(N, plan.T_lo, plan.T_hi,
                   tiles_per_chunk=plan.tiles_per_chunk,
                   max_labs=plan.max_labs)
    in_maps = []
    for c in range(NCORES):
        idx_all = np.concatenate([pack_idx16(plan.lo_idx[c]),
                                  pack_idx16(plan.hi_idx[c])], axis=1)
        ss_all = np.concatenate([plan.lo_ss[c], plan.hi_ss[c]], axis=0)
        ss_flat = np.ascontiguousarray(
            ss_all.transpose(1, 0, 2).reshape(P, -1))
        in_maps.append({"x": x, "idx16": idx_all, "ss": ss_flat})
    r1 = run_bass_kernel_spmd(nc1, in_maps, core_ids=list(range(NCORES)),
                              trace=trace)
    if r1.exec_time_ns:
        exec_ns.append(("L1", r1.exec_time_ns))
    tbls = [r1.results[c]["tbl"] for c in range(NCORES)]

    # host: assemble outputs
    rows = unscramble(tbls, plan, n_labels, C)
    x_out = np.zeros((N, C), np.float32)
    x_out[label_vals] = rows.astype(np.float32)
    eio = np.stack([cluster[src], cluster[dst]]).astype(np.int32)
    batch_out = np.zeros((N,), batch.dtype)
    batch_out[cluster] = batch
    kernel.last_exec_ns = exec_ns
    return x_out, eio, batch_out, cluster.astype(np.int32)
